# revision 18
# baseline (speedup 1.0000x reference)
"""Trainium2 Bass kernel for per-sample modulated+demodulated 3D conv.

Problem: x[B=8, CIN=128, 32,32,32], y[8,128], weight[128,128,3,3,3] (shared).
  w_b = weight * (1 + y[b,i]);  w_b *= rsqrt(sum_{i,k} w_b^2 + eps)  (per out-ch)
  out[b] = conv3d(x[b], w_b, same padding)

Sharding: data-parallel over batch, one sample per NeuronCore (8 cores).

Per core the conv uses Winograd F(2,3) along the W axis: the 3 kw-taps of
each (kd,kh) position become 4 transformed weights U_p applied to a
transformed input V_p (4 points per 2 output columns), cutting tensor-engine
columns 1.5x vs direct.  V is built on the Vector engine from stride-2
column pairs (adds only); the inverse transform (y0=M0+M1+M2, y1=M1-M2-M3)
runs on Scalar (PSUM evac with demod scale) + Vector with interleaved
stride-2 output writes.  Modulation (1+y_i) folds into U per-partition,
demodulation div[o] folds into the PSUM evacuation.  Matmul operands fp16.
"""

import sys

import numpy as np

try:
    import concourse.bass as bass
except ImportError:  # fresh grading dir: fall back to the repo checkout
    sys.path.insert(0, "/opt/trn_rl_repo")
    import concourse.bass as bass

import concourse.tile as tile
from concourse import bacc, mybir
from concourse.masks import make_identity

B, CIN, COUT, K = 8, 128, 128, 3
D = H = W = 32
T = K * K * K  # 27
NT = W // 2  # 16 winograd tiles along W
HP = H + 2  # padded rows (for kh shifts)
EPS = 1e-8
N_CORES = 8

FP32 = mybir.dt.float32
MM_DT = mybir.dt.float16

N_WARM = 10  # PE warmup dummy matmuls (HAM un-throttle)
VROT = 6  # rotating V-plane buffers

_CACHE = {}


def _g(kd, kh, p):
    return (kd * 3 + kh) * 4 + p


def _build_program():
    nc = bacc.Bacc()
    xv = nc.dram_tensor("x", [CIN, D, H, W], FP32, kind="ExternalInput")
    yv = nc.dram_tensor("y", [CIN, 1], FP32, kind="ExternalInput")
    # w is host-reordered to [o, t, i] with t = kd*9 + kh*3 + kw so each
    # (kd,kh) tap triple is one congruent contiguous DMA slice.  w2 is the
    # original [o, i, t] layout, used only for the demod-norm reduction.
    wv = nc.dram_tensor("w", [COUT, T, CIN], FP32, kind="ExternalInput")
    w2v = nc.dram_tensor("w2", [COUT, CIN, T], FP32, kind="ExternalInput")
    ov = nc.dram_tensor("out", [COUT, D, H, W], FP32, kind="ExternalOutput")

    AL = mybir.AluOpType

    with tile.TileContext(nc) as tc:
        with (
            tc.tile_pool(name="const", bufs=1) as const,
            tc.tile_pool(name="xnat", bufs=4) as xnat,
            tc.tile_pool(name="wtp", bufs=2) as wtp,
            tc.tile_pool(name="spool", bufs=2) as spool,
            tc.tile_pool(name="opool", bufs=3) as opool,
            tc.tile_pool(name="psum", bufs=6, space="PSUM") as psum,
            tc.tile_pool(name="psw", bufs=2, space="PSUM") as psw,
        ):
            # ---------------- x staging DMA (first: longest pole) ----------
            nat_tiles = [None] * D

            def stage_plane(p):
                nat = xnat.tile([CIN, H, W], FP32, tag="xnat", name="nat")
                nc.sync.dma_start(out=nat, in_=xv[:, p, :, :])
                nat_tiles[p] = nat

            stage_plane(0)
            stage_plane(1)

            # ---------------- PE warmup (HAM un-throttle from t~1us) -------
            wz = const.tile([128, 3, 128], MM_DT, tag="wz")
            nc.vector.memset(wz, 0.0)
            for i in range(N_WARM):
                pw = psw.tile([128, 3, 128], FP32, tag="psw", name="pw")
                nc.tensor.matmul(pw, wz[:, 0, :], wz, start=True, stop=True)

            # ---------------- weight + y DMA (per-tap-group slices) --------
            # wa[o, t, i]; group (kd,kh) covers t in [kd*9+kh*3, +3)
            wa = const.tile([COUT, T, CIN], FP32, tag="wa")
            for kd in (1, 2, 0):
                for kh in range(3):
                    t0 = kd * 9 + kh * 3
                    nc.sync.dma_start(
                        out=wa[:, t0 : t0 + 3, :], in_=wv[:, t0 : t0 + 3, :]
                    )
            w2a = const.tile([COUT, CIN, T], FP32, tag="w2a")
            nc.sync.dma_start(out=w2a, in_=w2v[:, :, :])
            ym = const.tile([CIN, 1], FP32, tag="ym")
            nc.sync.dma_start(out=ym, in_=yv[:, :])
            yrow = const.tile([1, CIN], FP32, tag="yrow")
            nc.sync.dma_start(out=yrow, in_=yv[:, 0])

            # (1+y) and 0.5*(1+y) per-partition columns
            ymp1 = const.tile([CIN, 1], FP32, tag="ymp1")
            nc.vector.tensor_scalar_add(ymp1, ym, 1.0)
            ymp1h = const.tile([CIN, 1], FP32, tag="ymp1h")
            nc.vector.tensor_scalar(ymp1h, ym, 1.0, 0.5, AL.add, AL.mult)

            ident = const.tile([128, 128], FP32, tag="ident")
            make_identity(nc, ident)

            # ---------------- V-plane buffers (H-pad borders zeroed once) --
            vslots = []
            for r in range(VROT):
                vt = const.tile([CIN, HP, 4, NT], MM_DT, tag=f"v{r}", name=f"v{r}")
                nc.vector.memset(vt[:, 0, :, :], 0.0)
                nc.vector.memset(vt[:, HP - 1, :, :], 0.0)
                vslots.append(vt)
            v_tiles = [None] * D

            def build_v(p):
                if nat_tiles[p] is None:
                    stage_plane(p)
                vt = vslots[p % VROT]
                na = nat_tiles[p]
                # interior rows 1..32 <- nat rows 0..31
                # v1[wt] = x[2wt] + x[2wt+1]
                nc.vector.tensor_add(
                    vt[:, 1 : HP - 1, 1, :], na[:, :, 0::2], na[:, :, 1::2]
                )
                # v2[wt] = x[2wt+1] - x[2wt]
                nc.vector.tensor_sub(
                    vt[:, 1 : HP - 1, 2, :], na[:, :, 1::2], na[:, :, 0::2]
                )
                # v0[wt>=1] = x[2wt-1] - x[2wt+1]; v0[0] = -x[1]
                nc.vector.tensor_sub(
                    vt[:, 1 : HP - 1, 0, 1:], na[:, :, 1:30:2], na[:, :, 3:32:2]
                )
                nc.vector.tensor_scalar_mul(
                    vt[:, 1 : HP - 1, 0, 0:1], na[:, :, 1:2], -1.0
                )
                # v3[wt<=14] = x[2wt] - x[2wt+2]; v3[15] = x[30]
                nc.vector.tensor_sub(
                    vt[:, 1 : HP - 1, 3, 0:15], na[:, :, 0:29:2], na[:, :, 2:31:2]
                )
                nc.vector.tensor_copy(
                    vt[:, 1 : HP - 1, 3, 15:16], na[:, :, 30:31]
                )
                v_tiles[p] = vt

            build_v(0)
            build_v(1)

            # ---------------- U weights (36 stationary tiles) --------------
            # U[:, g, :] = transformed, modulated weight for (kd, kh, p):
            #   t_kw[i,o] = w[o,i,(kd,kh,kw)]   (PE transpose)
            #   u0 = (1+y_i)*t0, u1 = .5*(1+y_i)*(t0+t1+t2),
            #   u2 = .5*(1+y_i)*(t0-t1+t2), u3 = (1+y_i)*t2
            U = const.tile([CIN, 36, COUT], MM_DT, tag="U")
            for kd in (1, 2, 0):  # planes 0/1 need kd 1,2 first
                for kh in range(3):
                    t = kd * 9 + kh * 3
                    pst = psw.tile([128, 3, 128], FP32, tag="psw", name="pst")
                    for kw in range(3):
                        nc.tensor.transpose(pst[:, kw, :], wa[:, t + kw, :], ident)
                    nc.scalar.activation(
                        out=U[:, _g(kd, kh, 0), :],
                        in_=pst[:, 0, :],
                        func=mybir.ActivationFunctionType.Copy,
                        scale=ymp1,
                    )
                    nc.scalar.activation(
                        out=U[:, _g(kd, kh, 3), :],
                        in_=pst[:, 2, :],
                        func=mybir.ActivationFunctionType.Copy,
                        scale=ymp1,
                    )
                    e0 = wtp.tile([CIN, COUT], FP32, tag="e0", name="e0")
                    e1 = wtp.tile([CIN, COUT], FP32, tag="e1", name="e1")
                    e2 = wtp.tile([CIN, COUT], FP32, tag="e2", name="e2")
                    nc.vector.tensor_scalar_mul(e0, pst[:, 0, :], ymp1h)
                    nc.vector.tensor_scalar_mul(e1, pst[:, 1, :], ymp1h)
                    nc.vector.tensor_scalar_mul(e2, pst[:, 2, :], ymp1h)
                    a = wtp.tile([CIN, COUT], FP32, tag="a", name="a")
                    nc.vector.tensor_add(a, e0, e2)
                    nc.vector.tensor_add(U[:, _g(kd, kh, 1), :], a, e1)
                    nc.vector.tensor_sub(U[:, _g(kd, kh, 2), :], a, e1)

            # ---------------- demod scale div[o] ---------------------------
            # div = rsqrt(sum_i (1+y_i)^2 * sum_t w[o,i,t]^2 + eps)
            # wsq on the (idle) scalar engine; reductions on vector, issued
            # after the U combos so they don't block the conv-critical path.
            wsq = const.tile([COUT, CIN, T], FP32, tag="wsq")
            nc.scalar.square(wsq, w2a)

            yp1row = const.tile([1, CIN], FP32, tag="yp1row")
            nc.vector.tensor_scalar_add(yp1row, yrow, 1.0)
            srow = const.tile([1, CIN], FP32, tag="srow")
            nc.vector.tensor_mul(srow, yp1row, yp1row)
            ones1 = const.tile([1, 128], FP32, tag="ones1")
            nc.vector.memset(ones1, 1.0)
            ps_s = psw.tile([128, 3, 128], FP32, tag="psw", name="ps_s")
            nc.tensor.matmul(ps_s[:, 0, :], ones1, srow, start=True, stop=True)

            q = const.tile([COUT, CIN], FP32, tag="q")
            nc.vector.tensor_reduce(q, wsq, mybir.AxisListType.X, AL.add)
            tq = const.tile([COUT, CIN], FP32, tag="tq")
            nc.vector.tensor_mul(tq, q, ps_s[:, 0, :])
            ncol = const.tile([COUT, 1], FP32, tag="ncol")
            nc.vector.reduce_sum(ncol, tq, axis=mybir.AxisListType.X)
            epst = const.tile([COUT, 1], FP32, tag="epst")
            nc.vector.memset(epst, EPS)
            sqn = const.tile([COUT, 1], FP32, tag="sqn")
            nc.scalar.activation(
                out=sqn,
                in_=ncol,
                func=mybir.ActivationFunctionType.Sqrt,
                bias=epst,
                scale=1.0,
            )
            div = const.tile([COUT, 1], FP32, tag="div")
            nc.vector.reciprocal(div, sqn)

            # ---------------- conv main loop -------------------------------
            for d in range(D):
                if d + 2 < D and nat_tiles[d + 2] is None:
                    stage_plane(d + 2)
                if d + 1 < D and v_tiles[d + 1] is None:
                    build_v(d + 1)

                taps = [
                    (kd, kh)
                    for kd in range(3)
                    if 0 <= d + kd - 1 < D
                    for kh in range(3)
                ]
                ms = []
                for p in range(4):
                    mp = psum.tile([COUT, H, NT], FP32, tag="m", name="mp")
                    for idx, (kd, kh) in enumerate(taps):
                        vp = v_tiles[d + kd - 1]
                        nc.tensor.matmul(
                            mp,
                            U[:, _g(kd, kh, p), :],
                            vp[:, kh : kh + H, p, :],
                            start=(idx == 0),
                            stop=(idx == len(taps) - 1),
                        )
                    ms.append(mp)

                ss = []
                for p in range(4):
                    sp = spool.tile([COUT, H, NT], FP32, tag=f"s{p}", name=f"s{p}")
                    nc.scalar.activation(
                        out=sp,
                        in_=ms[p],
                        func=mybir.ActivationFunctionType.Copy,
                        scale=div,
                    )
                    ss.append(sp)

                ot = opool.tile([COUT, H, W], FP32, tag="ot", name="ot")
                t01 = spool.tile([COUT, H, NT], FP32, tag="t01", name="t01")
                nc.vector.tensor_add(t01, ss[0], ss[1])
                nc.vector.tensor_add(ot[:, :, 0::2], t01, ss[2])
                t12 = spool.tile([COUT, H, NT], FP32, tag="t12", name="t12")
                nc.vector.tensor_sub(t12, ss[1], ss[2])
                nc.vector.tensor_sub(ot[:, :, 1::2], t12, ss[3])
                nc.sync.dma_start(out=ov[:, d, :, :], in_=ot)

    nc.compile()
    return nc


def _make_runner(nc):
    """Build the jitted 8-core executor once (mirrors
    bass2jax.run_bass_via_pjrt's multi-core path, but cacheable)."""
    import jax
    from jax.experimental.shard_map import shard_map
    from jax.sharding import Mesh, PartitionSpec

    from concourse import bass2jax

    bass2jax.install_neuronx_cc_hook()

    partition_name = (
        nc.partition_id_tensor.name if nc.partition_id_tensor else None
    )
    in_names, out_names, out_avals, zero_shapes = [], [], [], []
    for alloc in nc.m.functions[0].allocations:
        if not isinstance(alloc, mybir.MemoryLocationSet):
            continue
        name = alloc.memorylocations[0].name
        if alloc.kind == "ExternalInput":
            if name != partition_name:
                in_names.append(name)
        elif alloc.kind == "ExternalOutput":
            out_names.append(name)
            shape = tuple(alloc.tensor_shape)
            dtype = mybir.dt.np(alloc.dtype)
            out_avals.append(jax.core.ShapedArray(shape, dtype))
            zero_shapes.append((shape, dtype))
    n_params = len(in_names)
    n_outs = len(out_names)
    bind_in_names = in_names + out_names
    if partition_name is not None:
        bind_in_names = bind_in_names + [partition_name]
    bind_in_names = tuple(bind_in_names)
    donate = tuple(range(n_params, n_params + n_outs))

    def _body(*args):
        operands = list(args)
        if partition_name is not None:
            operands.append(bass2jax.partition_id_tensor())
        outs = bass2jax._bass_exec_p.bind(
            *operands,
            out_avals=tuple(out_avals),
            in_names=bind_in_names,
            out_names=tuple(out_names),
            lowering_input_output_aliases=(),
            sim_require_finite=True,
            sim_require_nnan=True,
            nc=nc,
        )
        return tuple(outs)

    devices = jax.devices()[:N_CORES]
    mesh = Mesh(np.asarray(devices), ("core",))
    in_specs = (PartitionSpec("core"),) * (n_params + n_outs)
    out_specs = (PartitionSpec("core"),) * n_outs
    sharded = jax.jit(
        shard_map(
            _body, mesh=mesh, in_specs=in_specs, out_specs=out_specs, check_rep=False
        ),
        donate_argnums=donate,
        keep_unused=True,
    )

    def run(in_maps):
        concat_in = [
            np.concatenate([np.asarray(m[n]) for m in in_maps], axis=0)
            for n in in_names
        ]
        concat_zeros = [
            np.zeros((N_CORES * s[0], *s[1:]), dt) for s, dt in zero_shapes
        ]
        out_arrs = sharded(*concat_in, *concat_zeros)
        return [
            {
                n: np.asarray(out_arrs[i]).reshape(N_CORES, *out_avals[i].shape)[c]
                for i, n in enumerate(out_names)
            }
            for c in range(N_CORES)
        ]

    return run


def kernel(x: np.ndarray, y: np.ndarray, weight: np.ndarray) -> np.ndarray:
    x = np.ascontiguousarray(np.asarray(x, dtype=np.float32))
    y = np.ascontiguousarray(np.asarray(y, dtype=np.float32))
    weight = np.ascontiguousarray(np.asarray(weight, dtype=np.float32))

    if "run" not in _CACHE:
        _CACHE["nc"] = _build_program()
        _CACHE["run"] = _make_runner(_CACHE["nc"])
    run = _CACHE["run"]

    wflat = np.ascontiguousarray(weight.reshape(COUT, CIN, T))
    wtap = np.ascontiguousarray(wflat.transpose(0, 2, 1))  # [o, t, i]
    in_maps = [
        {"x": x[b], "y": y[b].reshape(CIN, 1), "w": wtap, "w2": wflat}
        for b in range(B)
    ]
    results = run(in_maps)
    out = np.stack(
        [results[b]["out"].reshape(COUT, D, H, W) for b in range(B)], axis=0
    )
    return out


if __name__ == "__main__":
    rng = np.random.default_rng(0)
    x = rng.standard_normal((B, CIN, D, H, W), dtype=np.float32)
    y = rng.standard_normal((B, CIN), dtype=np.float32)
    w = rng.standard_normal((COUT, CIN, K, K, K), dtype=np.float32) * 0.017
    out = kernel(x=x, y=y, weight=w)
    print("out", out.shape, out.dtype, float(np.abs(out).max()))


# revision 19
# speedup vs baseline: 1.0927x; 1.0927x over previous
"""Trainium2 Bass kernel for per-sample modulated+demodulated 3D conv.

Problem: x[B=8, CIN=128, 32,32,32], y[8,128], weight[128,128,3,3,3] (shared).
  w_b = weight * (1 + y[b,i]);  w_b *= rsqrt(sum_{i,k} w_b^2 + eps)  (per out-ch)
  out[b] = conv3d(x[b], w_b, same padding)

Sharding: data-parallel over batch, one sample per NeuronCore (8 cores).

The conv uses Winograd F(2,3) along the W axis: the 3 kw-taps of each
(kd,kh) position become 4 transformed weights U_p applied to a transformed
input V_p (4 points per 2 output columns), cutting tensor-engine columns
1.5x vs direct.  V is built on the Vector engine from stride-2 column pairs
(adds only); the inverse transform (y0=M0+M1+M2, y1=M1-M2-M3) runs on
Scalar (PSUM evac with demod scale) + Vector with stride-2 output writes.

The tiny per-sample weight prep (modulation by 1+y, Winograd G-transform,
demod norm; ~0.1% of the FLOPs) is precomputed on the host in numpy and
DMA'd in as U[i, g, o] fp16 and div[o] fp32, so the tensor engine spends
the whole kernel on conv matmuls.
"""

import sys

import numpy as np

try:
    import concourse.bass as bass
except ImportError:  # fresh grading dir: fall back to the repo checkout
    sys.path.insert(0, "/opt/trn_rl_repo")
    import concourse.bass as bass

import concourse.tile as tile
from concourse import bacc, mybir

B, CIN, COUT, K = 8, 128, 128, 3
D = H = W = 32
T = K * K * K  # 27
NT = W // 2  # 16 winograd tiles along W
HP = H + 2  # padded rows (for kh shifts)
EPS = 1e-8
N_CORES = 8

FP32 = mybir.dt.float32
MM_DT = mybir.dt.float16

N_WARM = 20  # PE warmup dummy matmuls (HAM un-throttle)
VROT = 6  # rotating V-plane buffers

_CACHE = {}


def _g(kd, kh, p):
    return (kd * 3 + kh) * 4 + p


def _build_program():
    nc = bacc.Bacc()
    xv = nc.dram_tensor("x", [CIN, D, H, W], FP32, kind="ExternalInput")
    uv = nc.dram_tensor("U", [CIN, 36, COUT], MM_DT, kind="ExternalInput")
    dv = nc.dram_tensor("div", [COUT, 1], FP32, kind="ExternalInput")
    ov = nc.dram_tensor("out", [COUT, D, H, W], FP32, kind="ExternalOutput")

    with tile.TileContext(nc) as tc:
        with (
            tc.tile_pool(name="const", bufs=1) as const,
            tc.tile_pool(name="xnat", bufs=4) as xnat,
            tc.tile_pool(name="spool", bufs=2) as spool,
            tc.tile_pool(name="opool", bufs=3) as opool,
            tc.tile_pool(name="psum", bufs=6, space="PSUM") as psum,
            tc.tile_pool(name="psw", bufs=2, space="PSUM") as psw,
        ):
            # ---------------- input DMAs (x planes first: critical path) ---
            nat_tiles = [None] * D

            def stage_plane(p):
                nat = xnat.tile([CIN, H, W], FP32, tag="xnat", name="nat")
                nc.sync.dma_start(out=nat, in_=xv[:, p, :, :])
                nat_tiles[p] = nat

            stage_plane(0)
            stage_plane(1)

            U = const.tile([CIN, 36, COUT], MM_DT, tag="U")
            nc.sync.dma_start(out=U, in_=uv[:, :, :])
            div = const.tile([COUT, 1], FP32, tag="div")
            nc.sync.dma_start(out=div, in_=dv[:, :])

            # ---------------- PE warmup (HAM un-throttle from t~1us) -------
            wz = const.tile([128, 3, 128], MM_DT, tag="wz")
            nc.vector.memset(wz, 0.0)
            for i in range(N_WARM):
                pw = psw.tile([128, 3, 128], FP32, tag="psw", name="pw")
                nc.tensor.matmul(pw, wz[:, 0, :], wz, start=True, stop=True)

            # ---------------- V-plane buffers (H-pad borders zeroed once) --
            vslots = []
            for r in range(VROT):
                vt = const.tile([CIN, HP, 4, NT], MM_DT, tag=f"v{r}", name=f"v{r}")
                nc.vector.memset(vt[:, 0, :, :], 0.0)
                nc.vector.memset(vt[:, HP - 1, :, :], 0.0)
                vslots.append(vt)
            v_tiles = [None] * D

            def build_v(p):
                if nat_tiles[p] is None:
                    stage_plane(p)
                vt = vslots[p % VROT]
                na = nat_tiles[p]
                # interior rows 1..32 <- nat rows 0..31
                # v1[wt] = x[2wt] + x[2wt+1]
                nc.vector.tensor_add(
                    vt[:, 1 : HP - 1, 1, :], na[:, :, 0::2], na[:, :, 1::2]
                )
                # v2[wt] = x[2wt+1] - x[2wt]
                nc.vector.tensor_sub(
                    vt[:, 1 : HP - 1, 2, :], na[:, :, 1::2], na[:, :, 0::2]
                )
                # v0[wt>=1] = x[2wt-1] - x[2wt+1]; v0[0] = -x[1]
                nc.vector.tensor_sub(
                    vt[:, 1 : HP - 1, 0, 1:], na[:, :, 1:30:2], na[:, :, 3:32:2]
                )
                nc.vector.tensor_scalar_mul(
                    vt[:, 1 : HP - 1, 0, 0:1], na[:, :, 1:2], -1.0
                )
                # v3[wt<=14] = x[2wt] - x[2wt+2]; v3[15] = x[30]
                nc.vector.tensor_sub(
                    vt[:, 1 : HP - 1, 3, 0:15], na[:, :, 0:29:2], na[:, :, 2:31:2]
                )
                nc.vector.tensor_copy(
                    vt[:, 1 : HP - 1, 3, 15:16], na[:, :, 30:31]
                )
                v_tiles[p] = vt

            build_v(0)
            build_v(1)

            # ---------------- conv main loop -------------------------------
            for d in range(D):
                if d + 2 < D and nat_tiles[d + 2] is None:
                    stage_plane(d + 2)
                if d + 1 < D and v_tiles[d + 1] is None:
                    build_v(d + 1)

                taps = [
                    (kd, kh)
                    for kd in range(3)
                    if 0 <= d + kd - 1 < D
                    for kh in range(3)
                ]
                ms = []
                for p in range(4):
                    mp = psum.tile([COUT, H, NT], FP32, tag="m", name="mp")
                    for idx, (kd, kh) in enumerate(taps):
                        vp = v_tiles[d + kd - 1]
                        nc.tensor.matmul(
                            mp,
                            U[:, _g(kd, kh, p), :],
                            vp[:, kh : kh + H, p, :],
                            start=(idx == 0),
                            stop=(idx == len(taps) - 1),
                        )
                    ms.append(mp)

                ss = []
                for p in range(4):
                    sp = spool.tile([COUT, H, NT], FP32, tag=f"s{p}", name=f"s{p}")
                    nc.scalar.activation(
                        out=sp,
                        in_=ms[p],
                        func=mybir.ActivationFunctionType.Copy,
                        scale=div,
                    )
                    ss.append(sp)

                ot = opool.tile([COUT, H, W], FP32, tag="ot", name="ot")
                t01 = spool.tile([COUT, H, NT], FP32, tag="t01", name="t01")
                nc.vector.tensor_add(t01, ss[0], ss[1])
                nc.vector.tensor_add(ot[:, :, 0::2], t01, ss[2])
                t12 = spool.tile([COUT, H, NT], FP32, tag="t12", name="t12")
                nc.vector.tensor_sub(t12, ss[1], ss[2])
                nc.vector.tensor_sub(ot[:, :, 1::2], t12, ss[3])
                nc.sync.dma_start(out=ov[:, d, :, :], in_=ot)

    nc.compile()
    return nc


def _host_prep(y: np.ndarray, weight: np.ndarray):
    """Per-sample modulated Winograd-G weights U[b, i, g, o] (fp16) and demod
    scale div[b, o] (fp32) -- mirrors the reference exactly."""
    w5 = weight.reshape(COUT, CIN, K, K, K).astype(np.float64)
    wm = w5[None] * (y[:, None, :, None, None, None].astype(np.float64) + 1.0)
    dv = 1.0 / np.sqrt(np.sum(wm * wm, axis=(2, 3, 4, 5)) + EPS)  # [B, O]
    # U combos along kw: [B, O, I, kd, kh, 4]
    w0, w1, w2 = wm[..., 0], wm[..., 1], wm[..., 2]
    u = np.stack(
        [w0, 0.5 * (w0 + w1 + w2), 0.5 * (w0 - w1 + w2), w2], axis=-1
    )
    # -> [B, I, kd, kh, 4, O] -> [B, I, 36, O]
    u = np.transpose(u, (0, 2, 3, 4, 5, 1)).reshape(B, CIN, 36, COUT)
    return (
        np.ascontiguousarray(u.astype(np.float16)),
        np.ascontiguousarray(dv.astype(np.float32).reshape(B, COUT, 1)),
    )


def _make_runner(nc):
    """Build the jitted 8-core executor once (mirrors
    bass2jax.run_bass_via_pjrt's multi-core path, but cacheable)."""
    import jax
    from jax.experimental.shard_map import shard_map
    from jax.sharding import Mesh, PartitionSpec

    from concourse import bass2jax

    bass2jax.install_neuronx_cc_hook()

    partition_name = (
        nc.partition_id_tensor.name if nc.partition_id_tensor else None
    )
    in_names, out_names, out_avals, zero_shapes = [], [], [], []
    for alloc in nc.m.functions[0].allocations:
        if not isinstance(alloc, mybir.MemoryLocationSet):
            continue
        name = alloc.memorylocations[0].name
        if alloc.kind == "ExternalInput":
            if name != partition_name:
                in_names.append(name)
        elif alloc.kind == "ExternalOutput":
            out_names.append(name)
            shape = tuple(alloc.tensor_shape)
            dtype = mybir.dt.np(alloc.dtype)
            out_avals.append(jax.core.ShapedArray(shape, dtype))
            zero_shapes.append((shape, dtype))
    n_params = len(in_names)
    n_outs = len(out_names)
    bind_in_names = in_names + out_names
    if partition_name is not None:
        bind_in_names = bind_in_names + [partition_name]
    bind_in_names = tuple(bind_in_names)
    donate = tuple(range(n_params, n_params + n_outs))

    def _body(*args):
        operands = list(args)
        if partition_name is not None:
            operands.append(bass2jax.partition_id_tensor())
        outs = bass2jax._bass_exec_p.bind(
            *operands,
            out_avals=tuple(out_avals),
            in_names=bind_in_names,
            out_names=tuple(out_names),
            lowering_input_output_aliases=(),
            sim_require_finite=True,
            sim_require_nnan=True,
            nc=nc,
        )
        return tuple(outs)

    devices = jax.devices()[:N_CORES]
    mesh = Mesh(np.asarray(devices), ("core",))
    in_specs = (PartitionSpec("core"),) * (n_params + n_outs)
    out_specs = (PartitionSpec("core"),) * n_outs
    sharded = jax.jit(
        shard_map(
            _body, mesh=mesh, in_specs=in_specs, out_specs=out_specs, check_rep=False
        ),
        donate_argnums=donate,
        keep_unused=True,
    )

    def run(in_maps):
        concat_in = [
            np.concatenate([np.asarray(m[n]) for m in in_maps], axis=0)
            for n in in_names
        ]
        concat_zeros = [
            np.zeros((N_CORES * s[0], *s[1:]), dt) for s, dt in zero_shapes
        ]
        out_arrs = sharded(*concat_in, *concat_zeros)
        return [
            {
                n: np.asarray(out_arrs[i]).reshape(N_CORES, *out_avals[i].shape)[c]
                for i, n in enumerate(out_names)
            }
            for c in range(N_CORES)
        ]

    return run


def kernel(x: np.ndarray, y: np.ndarray, weight: np.ndarray) -> np.ndarray:
    x = np.ascontiguousarray(np.asarray(x, dtype=np.float32))
    y = np.ascontiguousarray(np.asarray(y, dtype=np.float32))
    weight = np.ascontiguousarray(np.asarray(weight, dtype=np.float32))

    if "run" not in _CACHE:
        _CACHE["nc"] = _build_program()
        _CACHE["run"] = _make_runner(_CACHE["nc"])
    run = _CACHE["run"]

    U, dv = _host_prep(y, weight)
    in_maps = [
        {"x": x[b], "U": U[b], "div": dv[b]} for b in range(B)
    ]
    results = run(in_maps)
    out = np.stack(
        [results[b]["out"].reshape(COUT, D, H, W) for b in range(B)], axis=0
    )
    return out


if __name__ == "__main__":
    rng = np.random.default_rng(0)
    x = rng.standard_normal((B, CIN, D, H, W), dtype=np.float32)
    y = rng.standard_normal((B, CIN), dtype=np.float32)
    w = rng.standard_normal((COUT, CIN, K, K, K), dtype=np.float32) * 0.017
    out = kernel(x=x, y=y, weight=w)
    print("out", out.shape, out.dtype, float(np.abs(out).max()))
